# revision 33
# baseline (speedup 1.0000x reference)
"""GQA causal attention on 8 TRN2 NeuronCores.

Reference (B=2, T=2048, D=2048, 32 q-heads, 8 kv-heads, dh=64):
    q = x@wq.T, k = x@wk.T, v = x@wv.T  (GQA expand k/v 4x)
    out = softmax(q@k.T/8 + causal) @ v, concat heads, @ wo.T

Sharding: tensor-parallel over heads; core c owns q-heads [4c,4c+4) and
kv-head c. No on-device collectives: the output projection is row-parallel
(each core multiplies its own 256 context features into a full-size
partial), and the host sums the 8 partials (the "all-reduce" of the
hint, performed at unshard time).

Per-core pipeline (PE compute in fp16; accumulation fp32 in PSUM):
  1. Warmup matmuls on a dummy tile keep the PE HAM clock warm until the
     first x strip + wq DMAs land (split across the two HWDGE queues:
     scalar carries wq + x-rows 64:128, sync carries x-rows 0:64 + wkv).
  2. x^T streamed in d-major so every contraction is partition-axis.
     Q^T proj packs 2 heads per matmul (M=128); K^T/V^T share one matmul.
  3. K^T duplicated to partitions 64:128 so the two heads of a pair run
     their score matmuls concurrently via PE row-tiling (K=64).
  4. V^T tiles are transposed into PV-stationary layout by the DMA XBAR
     (dma_start(transpose=True) on the sync queue) - no PE/PSUM involved.
  5. S^T[k,q] tiles from matmul -> exp on ACT (scores are O(3), no max
     subtraction needed) -> causal masking of the diagonal tiles by a
     precomputed 0/1 mask multiply on DVE.
  6. PV matmul with V augmented by a ones column: row 64 of the PSUM
     accumulator is the softmax denominator for free.  Both heads of a
     pair go through one matmul instruction (3D moving AP) so the vv
     stationary is loaded once per key tile.  The [65, 2QB] accumulator
     is drained by a single DVE copy so the (bufs=1) PSUM bank recycles
     fast for the next pair.
  7. Denominators for a pair land in rows 0:2 of a [2, QB] tile ->
     reciprocal -> ONE K=2 one-hot matmul broadcasts both head-halves to
     128 partitions (PSUM from the projection pool, so the PV pool never
     blocks on it) -> fused normalize into ctx^T fp16.
  8. Row-parallel out^T partial = wo_c^T-slices @ ctx^T in [128, QB]
     PSUM chunks, written per-oc2 (128KB granularity) as fp16
     [B, 2048, T]; the final block's DMAs alternate gpsimd/sync queues
     so the exposed tail drains at 2x bandwidth; host sums cores.
"""

import sys

sys.path.insert(0, "/opt/trn_rl_repo")

import numpy as np

import concourse.bass as bass  # noqa: F401
import concourse.mybir as mybir
import concourse.tile as tile
from concourse.masks import make_identity
from concourse import bacc
from concourse.bass_utils import run_bass_kernel_spmd

F32 = mybir.dt.float32
F16 = mybir.dt.float16
EXP = mybir.ActivationFunctionType.Exp

B, T, D = 2, 2048, 2048
H, KVH, DH = 32, 8, 64
NC = 8
HPC = H // NC  # 4 q-heads per core
PAIRS = HPC // 2
QB = 512
KB = 128
NJ = T // QB
NKK = T // KB
ND = D // 128
VW = DH + 1

_CACHE = {}


def _build():
    nc = bacc.Bacc("TRN2", target_bir_lowering=False, debug=False, num_devices=NC)

    # x strip-major: xH[b, j, p, n, q] = x^T[b, n*128+p, j*512+q] -> 16KB
    # contiguous per partition per strip (fast DMA)
    xT = nc.dram_tensor("xT", [B, NJ, 128, ND, QB], F16, kind="ExternalInput").ap()
    wqT = nc.dram_tensor("wqT", [128, ND, 256], F16, kind="ExternalInput").ap()
    wkvT = nc.dram_tensor("wkvT", [128, ND, 128], F16, kind="ExternalInput").ap()
    woT = nc.dram_tensor("woT", [128, 2, D], F16, kind="ExternalInput").ap()
    masks = nc.dram_tensor("masks", [128, 1, 128], F16, kind="ExternalInput").ap()
    oh2 = nc.dram_tensor("oh2", [33, 128], F16, kind="ExternalInput").ap()
    # out block-major: outH[b, tt, och, p, n, q] = out^T[b, och*512+n*128+p,
    # tt*512+q] -> 4KB contiguous runs per partition
    outT = nc.dram_tensor(
        "outT", [B, NJ, 4, 128, 4, QB], F16, kind="ExternalOutput"
    ).ap()

    with tile.TileContext(nc) as tc:
        with (
            tc.tile_pool(name="const", bufs=1) as constp,
            tc.tile_pool(name="xstrip", bufs=4) as xtp,
            tc.tile_pool(name="ktp", bufs=2) as ktp,
            tc.tile_pool(name="vvp", bufs=2) as vvp,
            tc.tile_pool(name="qtp", bufs=5) as qtp,
            tc.tile_pool(name="esp", bufs=6) as esp,
            tc.tile_pool(name="small", bufs=3) as smallp,
            tc.tile_pool(name="oup", bufs=4) as oup,
            tc.tile_pool(name="rrp", bufs=4) as rrp,
            tc.tile_pool(name="ctxlp", bufs=2) as ctxlp,
            tc.tile_pool(name="ofp", bufs=3) as ofp,
            tc.tile_pool(name="ps_s", bufs=2, space="PSUM") as psp,
            tc.tile_pool(name="ps_proj", bufs=2, space="PSUM") as prjp,
            tc.tile_pool(name="ps_o", bufs=1, space="PSUM") as pop,
        ):
            # ---- warmup: keep the PE clock warm until the first strip lands ----
            dummy = constp.tile([64, 64], F16)
            nc.gpsimd.memset(dummy[:], 0.0)
            warm = pop.tile([64, 64], F32, tag="po", name="warm")
            for _ in range(96):
                nc.tensor.matmul(warm[:], dummy[:], dummy[:], start=True, stop=True)

            # ---- constants, split across the two HWDGE queues so weight and
            # x loads run in parallel (scalar: wq + x rows 64:128; sync:
            # x rows 0:64 + wkv).  Whole-tensor DMAs keep per-partition
            # runs >= 4KB (fast descriptors). ----
            wq_sb = constp.tile([128, ND, 256], F16)
            nc.scalar.dma_start(out=wq_sb[:, 0:2, :], in_=wqT[:, 0:2, :])
            nc.scalar.dma_start(out=wq_sb[:, 2:16, :], in_=wqT[:, 2:16, :])
            wkv_sb = constp.tile([128, ND, 128], F16)
            nc.scalar.dma_start(out=wkv_sb[:], in_=wkvT[:])
            mask_sb = constp.tile([128, 1, 128], F16)
            nc.scalar.dma_start(out=mask_sb[:], in_=masks[:])
            # one-hot: row 0 -> out partitions 0:64, row 32 -> 64:128
            onehot2 = constp.tile([33, 128], F16)
            nc.scalar.dma_start(out=onehot2[:], in_=oh2[:, :])
            ident = constp.tile([128, 128], F32)
            make_identity(nc, ident[:])
            wo_sb = constp.tile([128, 2, D], F16)
            wo_loaded = [False]

            def load_wo():
                if not wo_loaded[0]:
                    nc.scalar.dma_start(out=wo_sb[:], in_=woT[:])
                    wo_loaded[0] = True

            fp_units = []  # pending (b, tt, och) output-projection units

            def emit_fp_unit():
                # row-parallel output projection, one och chunk of a block
                if not fp_units:
                    return
                b, tt, och = fp_units.pop(0)
                load_wo()
                tail = b == B - 1 and tt == NJ - 1
                ctxl = ctxls[b]
                of = ofp.tile([128, 4, QB], F16, tag="of")
                for oc2 in range(4):
                    pf = prjp.tile([128, QB], F32, tag="proj", name="pf")
                    oc0 = (och * 4 + oc2) * 128
                    for g in range(2):
                        nc.tensor.matmul(
                            pf[:],
                            wo_sb[:, g, oc0 : oc0 + 128],
                            ctxl[:, g, tt * QB : (tt + 1) * QB],
                            start=(g == 0),
                            stop=(g == 1),
                        )
                    ofv = of[:, oc2, :]
                    if oc2 == 1:
                        nc.scalar.copy(ofv, pf[:])
                    else:
                        nc.vector.tensor_copy(ofv, pf[:])
                    if tail and oc2 == 1:
                        # tail block: chunks drain on three queues in parallel
                        # so the exposed final DMA is cut to a third
                        nc.sync.dma_start(
                            out=outT[b, tt, och][:, 0:2, :], in_=of[:, 0:2, :]
                        )
                    if tail and oc2 == 2:
                        nc.scalar.dma_start(
                            out=outT[b, tt, och][:, 2:3, :], in_=of[:, 2:3, :]
                        )
                if tail:
                    nc.gpsimd.dma_start(
                        out=outT[b, tt, och][:, 3:4, :], in_=of[:, 3:4, :]
                    )
                else:
                    nc.gpsimd.dma_start(out=outT[b, tt, och], in_=of[:])

            ctxls = []
            defer_norm = []
            for b in range(B):
                kt = ktp.tile([128, T], F16, tag="kt")  # K^T on both halves
                vv = vvp.tile([128, NKK * VW], F16, tag="vv")  # [V | 1] tiles
                nc.vector.memset(
                    vv[:].rearrange("p (n c) -> p n c", c=VW)[:, :, DH : DH + 1], 1.0
                )
                ctxl = ctxlp.tile([128, 2, T], F16, tag="ctxl")  # normalized ctx^T
                ctxls.append(ctxl)

                xts = []
                for j in range(NJ):
                    xt = xtp.tile([128, ND, QB], F16, tag="xstrip", name=f"xt{b}{j}")
                    xsrc = xT[b, j]
                    bounds = [0, 2, 8, 16] if (b == 0 and j == 0) else [0, 16]
                    for lo, hi in zip(bounds[:-1], bounds[1:]):
                        nc.sync.dma_start(
                            out=xt[:, lo:hi, :], in_=xsrc[:, lo:hi, :]
                        )
                    xts.append(xt)

                for j in range(NJ):
                    xt = xts[j]
                    nkk = 4 * (j + 1)
                    last_j = b == B - 1 and j == NJ - 1
                    qts = []

                    def q_proj(p):
                        pq = prjp.tile([128, QB], F32, tag="proj")
                        for dk in range(ND):
                            nc.tensor.matmul(
                                pq[:],
                                wq_sb[:, dk, p * 128 : (p + 1) * 128],
                                xt[:, dk, :],
                                start=(dk == 0),
                                stop=(dk == ND - 1),
                            )
                        qt = qtp.tile([128, QB], F16, tag="qt")
                        if p == 0:
                            nc.scalar.copy(qt[:], pq[:])
                        else:
                            nc.vector.tensor_copy(qt[:], pq[:])
                        qts.append(qt)

                    def kv_proj():
                        # K^T rows 0:64, V^T rows 64:128 in one accumulation
                        pkv = prjp.tile([128, QB], F32, tag="proj")
                        for dk in range(ND):
                            nc.tensor.matmul(
                                pkv[:],
                                wkv_sb[:, dk, :],
                                xt[:, dk, :],
                                start=(dk == 0),
                                stop=(dk == ND - 1),
                            )
                        # rows 0:64 partition-aligned -> ACT engine, so the
                        # DVE queue only carries the shifted duplicate and the
                        # pre-seeded score matmuls unblock sooner
                        nc.scalar.copy(kt[0:64, j * QB : (j + 1) * QB], pkv[0:64, :])
                        nc.vector.tensor_copy(
                            kt[64:128, j * QB : (j + 1) * QB], pkv[0:64, :]
                        )
                        vt_sb = smallp.tile([64, QB], F32, tag="vt")
                        nc.vector.tensor_copy(vt_sb[:], pkv[64:128, :])
                        return vt_sb

                    def v_transp(vt_sb):
                        for i in range(4):
                            pvt = prjp.tile([128, 64], F32, tag="proj")
                            nc.tensor.transpose(
                                pvt[:],
                                vt_sb[:, i * 128 : (i + 1) * 128],
                                ident[0:64, 0:64],
                            )
                            kk = 4 * j + i
                            nc.vector.tensor_copy(vv[:, kk * VW : kk * VW + DH], pvt[:])

                    def emit_scores(kk, p):
                        m = kk - 4 * j
                        q0 = max(0, m) * KB  # masked-out query prefix
                        ps = psp.tile([128, 2 * QB], F32, tag="ps", name="ps")
                        nc.tensor.matmul(
                            ps[:, q0:QB],
                            kt[0:64, kk * KB : (kk + 1) * KB],
                            qts[p][0:64, q0:QB],
                            start=True,
                            stop=True,
                        )
                        nc.tensor.matmul(
                            ps[:, QB + q0 : 2 * QB],
                            kt[64:128, kk * KB : (kk + 1) * KB],
                            qts[p][64:128, q0:QB],
                            start=True,
                            stop=True,
                            tile_position=(64, 0),
                        )
                        es = esp.tile([128, 2 * QB], F16, tag="es", name="es")
                        if q0 == 0:
                            nc.scalar.activation(es[:], ps[:], EXP)
                        else:
                            # one strided activation over both head blocks
                            nc.scalar.activation(
                                es[:].rearrange("p (h q) -> p h q", q=QB)[:, :, q0:QB],
                                ps[:].rearrange("p (h q) -> p h q", q=QB)[:, :, q0:QB],
                                EXP,
                            )
                        if m >= 0:
                            # only the 128-wide diagonal square needs the
                            # causal mask; columns beyond it are unmasked
                            esv = es[:].rearrange("p (h q) -> p h q", q=QB)[
                                :, :, q0 : q0 + 128
                            ]
                            nc.vector.tensor_mul(
                                esv,
                                esv,
                                mask_sb[:].broadcast_to([128, 2, 128]),
                            )
                        return es, q0

                    def make_ir2(src):
                        # reciprocal of both head denominators (rows 0 and 32)
                        rr2 = rrp.tile([33, QB], F32, tag="rr2")
                        nc.vector.tensor_copy(rr2[0:1, :], src[64:65, 0:QB])
                        nc.vector.tensor_copy(rr2[32:33, :], src[64:65, QB : 2 * QB])
                        ir2f = rrp.tile([33, QB], F32, tag="ir2f")
                        nc.vector.reciprocal_approx_fast(ir2f[:], rr2[:])
                        ir2 = rrp.tile([33, QB], F16, tag="ir2")
                        nc.vector.tensor_copy(ir2[:], ir2f[:])
                        return ir2

                    def normalize(p, ous_l, j_l, ctxl_l, pbpool, ir2=None):
                        # PE tile-broadcast of the reciprocals per head half
                        if ir2 is None:
                            ir2 = make_ir2(ous_l[p])
                        pb = pbpool.tile(
                            [128, QB],
                            F32,
                            tag="po" if pbpool is pop else "proj",
                            name="pb",
                        )
                        nc.tensor.matmul(
                            pb[0:64, :],
                            onehot2[0:1, 0:64],
                            ir2[0:1, :],
                            start=True,
                            stop=True,
                            tile_position=(0, 0),
                        )
                        nc.tensor.matmul(
                            pb[64:128, :],
                            onehot2[32:33, 64:128],
                            ir2[32:33, :],
                            start=True,
                            stop=True,
                            tile_position=(32, 64),
                        )
                        for hh in range(2):
                            nc.vector.tensor_mul(
                                ctxl_l[
                                    64 * hh : 64 * hh + 64,
                                    p,
                                    j_l * QB : (j_l + 1) * QB,
                                ],
                                ous_l[p][0:64, hh * QB : (hh + 1) * QB],
                                pb[64 * hh : 64 * hh + 64, :],
                            )

                    pre = None
                    if b == 0 and j == 0:
                        # first block: wq lands before wkv on the scalar queue,
                        # so keep the Q-first order for the DMA cascade
                        q_proj(0)
                        q_proj(1)
                        vt_sb = kv_proj()
                        v_transp(vt_sb)
                    else:
                        # steady state: K first so the kt->score->exp chain of
                        # pair 0 fills while Q pair 1 still streams on the PE
                        vt_sb = kv_proj()
                        q_proj(0)
                        pre = [emit_scores(0, 0), emit_scores(1, 0)]
                        q_proj(1)
                        v_transp(vt_sb)
                    # the previous block's deferred pair-1 normalize lands
                    # here, AFTER this block's critical kt/qt copies and
                    # pre-seeded scores, so it no longer blocks them on DVE
                    for fn in defer_norm:
                        fn()
                    defer_norm.clear()

                    for _ in range(4):
                        emit_fp_unit()

                    # attention for this query block
                    ous = []
                    for p in range(PAIRS):
                        po = pop.tile([65, 2 * QB], F32, tag="po")
                        if p == 0 and pre is not None:
                            pipe = pre
                        else:
                            pipe = [emit_scores(0, p)]
                            if nkk > 1:
                                pipe.append(emit_scores(1, p))
                        for kk in range(nkk):
                            es_cur, q0 = pipe.pop(0)
                            if kk + 2 < nkk:
                                pipe.append(emit_scores(kk + 2, p))
                            for hh in range(2):
                                nc.tensor.matmul(
                                    po[0:65, hh * QB + q0 : (hh + 1) * QB],
                                    vv[:, kk * VW : (kk + 1) * VW],
                                    es_cur[:, hh * QB + q0 : (hh + 1) * QB],
                                    start=(kk == 0),
                                    stop=(kk == nkk - 1),
                                )
                        # single-copy PSUM drain: rows 0:64 ctx, row 64 denom
                        if last_j:
                            # final block: reciprocal chain straight from PSUM
                            # before the big drain copy, so the broadcast
                            # matmul overlaps the drain on the PE
                            ir2_t = make_ir2(po)
                        ou = oup.tile([65, 2 * QB], F32, tag="ou")
                        nc.vector.tensor_copy(ou[:], po[:])
                        ous.append(ou)
                        if last_j:
                            # pb from the projection pool: no aliasing wait on
                            # po, so the tail output projection starts early
                            normalize(p, ous, j, ctxl, prjp, ir2=ir2_t)

                    if not last_j:
                        normalize(0, ous, j, ctxl, pop)
                        defer_norm.append(
                            lambda o=ous, jj=j, cl=ctxl: normalize(1, o, jj, cl, pop)
                        )

                    fp_units.extend((b, j, och) for och in range(4))
            while fp_units:
                emit_fp_unit()
    nc.finalize()
    return nc


def _prep_in_maps(x, wq, wk, wv, wo):
    # strip-major x: [B, NJ, 128, ND, QB], 16KB contiguous per partition/strip
    xT = (
        x.transpose(0, 2, 1)
        .reshape(B, ND, 128, NJ, QB)
        .transpose(0, 3, 2, 1, 4)
        .astype(np.float16)
    )
    xT = np.ascontiguousarray(xT)
    k_idx = np.arange(128)[:, None]
    q_idx = np.arange(QB)[None, :]
    masks = np.stack(
        [(128 * m + k_idx <= q_idx).astype(np.float16) for m in range(4)]
    )
    oh2 = np.zeros((33, 128), np.float16)
    oh2[0, 0:64] = 1.0
    oh2[32, 64:128] = 1.0
    # single 128x128 diagonal-square mask (same for every diagonal tile)
    masks_h = np.ascontiguousarray(masks[0, :, 0:128].reshape(128, 1, 128))

    def pack_pnm(w):  # [D_in, M] -> [128 p, ND n, M]
        return np.ascontiguousarray(
            w.reshape(ND, 128, w.shape[1]).transpose(1, 0, 2)
        )

    in_maps = []
    for c in range(NC):
        wq_c = (wq[c * 256 : (c + 1) * 256] * np.float32(DH ** -0.5)).astype(np.float16)
        wkv_c = np.concatenate(
            [wk[c * DH : (c + 1) * DH], wv[c * DH : (c + 1) * DH]], axis=0
        ).astype(np.float16)
        wo_c = wo[:, c * 256 : (c + 1) * 256].astype(np.float16)  # [2048, 256]
        woT_c = wo_c.T  # [256, 2048]
        wo_h = np.ascontiguousarray(
            woT_c.reshape(2, 128, D).transpose(1, 0, 2)
        )  # [128 p, 2 g, D]
        in_maps.append(
            {
                "xT": xT,
                "wqT": pack_pnm(np.ascontiguousarray(wq_c.T)),
                "wkvT": pack_pnm(np.ascontiguousarray(wkv_c.T)),
                "woT": wo_h,
                "masks": masks_h,
                "oh2": oh2,
            }
        )
    return in_maps


def run(inputs, trace=False, trace_kwargs=None):
    if "nc" not in _CACHE:
        _CACHE["nc"] = _build()
    nc = _CACHE["nc"]
    in_maps = _prep_in_maps(
        np.asarray(inputs["x"], np.float32),
        np.asarray(inputs["wq"], np.float32),
        np.asarray(inputs["wk"], np.float32),
        np.asarray(inputs["wv"], np.float32),
        np.asarray(inputs["wo"], np.float32),
    )
    res = run_bass_kernel_spmd(
        nc,
        in_maps,
        core_ids=list(range(NC)),
        trace=trace,
        **(trace_kwargs or {}),
    )
    acc = np.zeros((B, NJ, 4, 128, 4, QB), np.float32)
    for r in res.results:
        acc += r["outT"]
    # [B, tt, och, p, n, q] -> [B, d=(och,n,p), t=(tt,q)] -> [B, T, D]
    full = acc.transpose(0, 2, 4, 3, 1, 5).reshape(B, D, T).transpose(0, 2, 1)
    return np.ascontiguousarray(full), res


def kernel(**inputs) -> np.ndarray:
    out, _ = run(inputs, trace=False)
    return out


# revision 34
# speedup vs baseline: 1.0206x; 1.0206x over previous
"""GQA causal attention on 8 TRN2 NeuronCores.

Reference (B=2, T=2048, D=2048, 32 q-heads, 8 kv-heads, dh=64):
    q = x@wq.T, k = x@wk.T, v = x@wv.T  (GQA expand k/v 4x)
    out = softmax(q@k.T/8 + causal) @ v, concat heads, @ wo.T

Sharding: tensor-parallel over heads; core c owns q-heads [4c,4c+4) and
kv-head c. No on-device collectives: the output projection is row-parallel
(each core multiplies its own 256 context features into a full-size
partial), and the host sums the 8 partials (the "all-reduce" of the
hint, performed at unshard time).

Per-core pipeline (PE compute in fp16; accumulation fp32 in PSUM):
  1. Warmup matmuls on a dummy tile keep the PE HAM clock warm until the
     first x strip + wq DMAs land (split across the two HWDGE queues:
     scalar carries wq + x-rows 64:128, sync carries x-rows 0:64 + wkv).
  2. x^T streamed in d-major so every contraction is partition-axis.
     Q^T proj packs 2 heads per matmul (M=128); K^T/V^T share one matmul.
  3. K^T duplicated to partitions 64:128 so the two heads of a pair run
     their score matmuls concurrently via PE row-tiling (K=64).
  4. V^T tiles are transposed into PV-stationary layout by the DMA XBAR
     (dma_start(transpose=True) on the sync queue) - no PE/PSUM involved.
  5. S^T[k,q] tiles from matmul -> exp on ACT (scores are O(3), no max
     subtraction needed) -> causal masking of the diagonal tiles by a
     precomputed 0/1 mask multiply on DVE.
  6. PV matmul with V augmented by a ones column: row 64 of the PSUM
     accumulator is the softmax denominator for free.  Both heads of a
     pair go through one matmul instruction (3D moving AP) so the vv
     stationary is loaded once per key tile.  The [65, 2QB] accumulator
     is drained by a single DVE copy so the (bufs=1) PSUM bank recycles
     fast for the next pair.
  7. Denominators for a pair land in rows 0:2 of a [2, QB] tile ->
     reciprocal -> ONE K=2 one-hot matmul broadcasts both head-halves to
     128 partitions (PSUM from the projection pool, so the PV pool never
     blocks on it) -> fused normalize into ctx^T fp16.
  8. Row-parallel out^T partial = wo_c^T-slices @ ctx^T in [128, QB]
     PSUM chunks, written per-oc2 (128KB granularity) as fp16
     [B, 2048, T]; the final block's DMAs alternate gpsimd/sync queues
     so the exposed tail drains at 2x bandwidth; host sums cores.
"""

import sys

sys.path.insert(0, "/opt/trn_rl_repo")

import numpy as np

import concourse.bass as bass  # noqa: F401
import concourse.mybir as mybir
import concourse.tile as tile
from concourse.masks import make_identity
from concourse import bacc
from concourse.bass_utils import run_bass_kernel_spmd

F32 = mybir.dt.float32
F16 = mybir.dt.float16
EXP = mybir.ActivationFunctionType.Exp

B, T, D = 2, 2048, 2048
H, KVH, DH = 32, 8, 64
NC = 8
HPC = H // NC  # 4 q-heads per core
PAIRS = HPC // 2
QB = 512
KB = 128
NJ = T // QB
NKK = T // KB
ND = D // 128
VW = DH + 1

_CACHE = {}


def _build():
    nc = bacc.Bacc("TRN2", target_bir_lowering=False, debug=False, num_devices=NC)

    # x strip-major: xH[b, j, p, n, q] = x^T[b, n*128+p, j*512+q] -> 16KB
    # contiguous per partition per strip (fast DMA)
    xT = nc.dram_tensor("xT", [B, NJ, 128, ND, QB], F16, kind="ExternalInput").ap()
    wqT = nc.dram_tensor("wqT", [128, ND, 256], F16, kind="ExternalInput").ap()
    wkvT = nc.dram_tensor("wkvT", [128, ND, 128], F16, kind="ExternalInput").ap()
    woT = nc.dram_tensor("woT", [128, 2, D], F16, kind="ExternalInput").ap()
    masks = nc.dram_tensor("masks", [128, 1, 128], F16, kind="ExternalInput").ap()
    oh2 = nc.dram_tensor("oh2", [33, 128], F16, kind="ExternalInput").ap()
    # out block-major: outH[b, tt, och, p, n, q] = out^T[b, och*512+n*128+p,
    # tt*512+q] -> 4KB contiguous runs per partition
    outT = nc.dram_tensor(
        "outT", [B, NJ, 4, 128, 4, QB], F16, kind="ExternalOutput"
    ).ap()

    with tile.TileContext(nc) as tc:
        with (
            tc.tile_pool(name="const", bufs=1) as constp,
            tc.tile_pool(name="xstrip", bufs=4) as xtp,
            tc.tile_pool(name="ktp", bufs=2) as ktp,
            tc.tile_pool(name="vvp", bufs=2) as vvp,
            tc.tile_pool(name="qtp", bufs=5) as qtp,
            tc.tile_pool(name="esp", bufs=6) as esp,
            tc.tile_pool(name="small", bufs=3) as smallp,
            tc.tile_pool(name="oup", bufs=4) as oup,
            tc.tile_pool(name="rrp", bufs=4) as rrp,
            tc.tile_pool(name="ctxlp", bufs=2) as ctxlp,
            tc.tile_pool(name="ofp", bufs=3) as ofp,
            tc.tile_pool(name="ps_s", bufs=2, space="PSUM") as psp,
            tc.tile_pool(name="ps_proj", bufs=2, space="PSUM") as prjp,
            tc.tile_pool(name="ps_o", bufs=1, space="PSUM") as pop,
        ):
            # ---- warmup: keep the PE clock warm until the first strip lands ----
            dummy = constp.tile([64, 64], F16)
            nc.gpsimd.memset(dummy[:], 0.0)
            warm = pop.tile([64, 64], F32, tag="po", name="warm")
            for _ in range(96):
                nc.tensor.matmul(warm[:], dummy[:], dummy[:], start=True, stop=True)

            # ---- constants, split across the two HWDGE queues so weight and
            # x loads run in parallel (scalar: wq + x rows 64:128; sync:
            # x rows 0:64 + wkv).  Whole-tensor DMAs keep per-partition
            # runs >= 4KB (fast descriptors). ----
            wq_sb = constp.tile([128, ND, 256], F16)
            nc.scalar.dma_start(out=wq_sb[:, 0:2, :], in_=wqT[:, 0:2, :])
            nc.scalar.dma_start(out=wq_sb[:, 2:16, :], in_=wqT[:, 2:16, :])
            wkv_sb = constp.tile([128, ND, 128], F16)
            nc.scalar.dma_start(out=wkv_sb[:], in_=wkvT[:])
            mask_sb = constp.tile([128, 1, 128], F16)
            nc.scalar.dma_start(out=mask_sb[:], in_=masks[:])
            # one-hot: row 0 -> out partitions 0:64, row 32 -> 64:128
            onehot2 = constp.tile([33, 128], F16)
            nc.scalar.dma_start(out=onehot2[:], in_=oh2[:, :])
            ident = constp.tile([128, 128], F32)
            make_identity(nc, ident[:])
            wo_sb = constp.tile([128, 2, D], F16)
            wo_loaded = [False]

            def load_wo():
                if not wo_loaded[0]:
                    nc.scalar.dma_start(out=wo_sb[:], in_=woT[:])
                    wo_loaded[0] = True

            fp_units = []  # pending (b, tt, och) output-projection units

            def emit_fp_unit():
                # row-parallel output projection, one och chunk of a block
                if not fp_units:
                    return
                b, tt, och = fp_units.pop(0)
                load_wo()
                tail = b == B - 1 and tt == NJ - 1
                ctxl = ctxls[b]
                of = ofp.tile([128, 4, QB], F16, tag="of")
                for oc2 in range(4):
                    pf = prjp.tile([128, QB], F32, tag="proj", name="pf")
                    oc0 = (och * 4 + oc2) * 128
                    for g in range(2):
                        nc.tensor.matmul(
                            pf[:],
                            wo_sb[:, g, oc0 : oc0 + 128],
                            ctxl[:, g, tt * QB : (tt + 1) * QB],
                            start=(g == 0),
                            stop=(g == 1),
                        )
                    ofv = of[:, oc2, :]
                    if oc2 == 1:
                        nc.scalar.copy(ofv, pf[:])
                    else:
                        nc.vector.tensor_copy(ofv, pf[:])
                    if tail and oc2 == 1:
                        # tail block: chunks drain on three queues in parallel
                        # so the exposed final DMA is cut to a third
                        nc.sync.dma_start(
                            out=outT[b, tt, och][:, 0:2, :], in_=of[:, 0:2, :]
                        )
                    if tail and oc2 == 2:
                        nc.scalar.dma_start(
                            out=outT[b, tt, och][:, 2:3, :], in_=of[:, 2:3, :]
                        )
                if tail:
                    nc.gpsimd.dma_start(
                        out=outT[b, tt, och][:, 3:4, :], in_=of[:, 3:4, :]
                    )
                else:
                    nc.gpsimd.dma_start(out=outT[b, tt, och], in_=of[:])

            ctxls = []
            defer_norm = []
            for b in range(B):
                kt = ktp.tile([128, T], F16, tag="kt")  # K^T on both halves
                vv = vvp.tile([128, NKK * VW], F16, tag="vv")  # [V | 1] tiles
                nc.vector.memset(
                    vv[:].rearrange("p (n c) -> p n c", c=VW)[:, :, DH : DH + 1], 1.0
                )
                ctxl = ctxlp.tile([128, 2, T], F16, tag="ctxl")  # normalized ctx^T
                ctxls.append(ctxl)

                xts = []
                for j in range(NJ):
                    xt = xtp.tile([128, ND, QB], F16, tag="xstrip", name=f"xt{b}{j}")
                    xsrc = xT[b, j]
                    bounds = [0, 2, 8, 16] if (b == 0 and j == 0) else [0, 16]
                    for lo, hi in zip(bounds[:-1], bounds[1:]):
                        nc.sync.dma_start(
                            out=xt[:, lo:hi, :], in_=xsrc[:, lo:hi, :]
                        )
                    xts.append(xt)

                for j in range(NJ):
                    xt = xts[j]
                    nkk = 4 * (j + 1)
                    last_j = b == B - 1 and j == NJ - 1
                    qts = []

                    def q_proj(p):
                        pq = prjp.tile([128, QB], F32, tag="proj")
                        for dk in range(ND):
                            nc.tensor.matmul(
                                pq[:],
                                wq_sb[:, dk, p * 128 : (p + 1) * 128],
                                xt[:, dk, :],
                                start=(dk == 0),
                                stop=(dk == ND - 1),
                            )
                        qt = qtp.tile([128, QB], F16, tag="qt")
                        if p == 0:
                            nc.scalar.copy(qt[:], pq[:])
                        else:
                            nc.vector.tensor_copy(qt[:], pq[:])
                        qts.append(qt)

                    def kv_proj():
                        # K^T rows 0:64, V^T rows 64:128 in one accumulation
                        pkv = prjp.tile([128, QB], F32, tag="proj")
                        for dk in range(ND):
                            nc.tensor.matmul(
                                pkv[:],
                                wkv_sb[:, dk, :],
                                xt[:, dk, :],
                                start=(dk == 0),
                                stop=(dk == ND - 1),
                            )
                        # rows 0:64 partition-aligned -> ACT engine, so the
                        # DVE queue only carries the shifted duplicate and the
                        # pre-seeded score matmuls unblock sooner
                        nc.scalar.copy(kt[0:64, j * QB : (j + 1) * QB], pkv[0:64, :])
                        nc.vector.tensor_copy(
                            kt[64:128, j * QB : (j + 1) * QB], pkv[0:64, :]
                        )
                        vt_sb = smallp.tile([64, QB], F32, tag="vt")
                        nc.vector.tensor_copy(vt_sb[:], pkv[64:128, :])
                        return vt_sb

                    def v_transp(vt_sb):
                        for i in range(4):
                            pvt = prjp.tile([128, 64], F32, tag="proj")
                            nc.tensor.transpose(
                                pvt[:],
                                vt_sb[:, i * 128 : (i + 1) * 128],
                                ident[0:64, 0:64],
                            )
                            kk = 4 * j + i
                            nc.vector.tensor_copy(vv[:, kk * VW : kk * VW + DH], pvt[:])

                    def emit_scores(kk, p):
                        m = kk - 4 * j
                        q0 = max(0, m) * KB  # masked-out query prefix
                        ps = psp.tile([128, 2 * QB], F32, tag="ps", name="ps")
                        nc.tensor.matmul(
                            ps[:, q0:QB],
                            kt[0:64, kk * KB : (kk + 1) * KB],
                            qts[p][0:64, q0:QB],
                            start=True,
                            stop=True,
                        )
                        nc.tensor.matmul(
                            ps[:, QB + q0 : 2 * QB],
                            kt[64:128, kk * KB : (kk + 1) * KB],
                            qts[p][64:128, q0:QB],
                            start=True,
                            stop=True,
                            tile_position=(64, 0),
                        )
                        es = esp.tile([128, 2 * QB], F16, tag="es", name="es")
                        if q0 == 0:
                            nc.scalar.activation(es[:], ps[:], EXP)
                        else:
                            # one strided activation over both head blocks
                            nc.scalar.activation(
                                es[:].rearrange("p (h q) -> p h q", q=QB)[:, :, q0:QB],
                                ps[:].rearrange("p (h q) -> p h q", q=QB)[:, :, q0:QB],
                                EXP,
                            )
                        if m >= 0:
                            # only the 128-wide diagonal square needs the
                            # causal mask; columns beyond it are unmasked
                            esv = es[:].rearrange("p (h q) -> p h q", q=QB)[
                                :, :, q0 : q0 + 128
                            ]
                            nc.vector.tensor_mul(
                                esv,
                                esv,
                                mask_sb[:].broadcast_to([128, 2, 128]),
                            )
                        return es, q0

                    def make_ir2(src):
                        # reciprocal of both head denominators (rows 0 and 32)
                        rr2 = rrp.tile([33, QB], F32, tag="rr2")
                        nc.vector.tensor_copy(rr2[0:1, :], src[64:65, 0:QB])
                        nc.vector.tensor_copy(rr2[32:33, :], src[64:65, QB : 2 * QB])
                        ir2f = rrp.tile([33, QB], F32, tag="ir2f")
                        nc.vector.reciprocal_approx_fast(ir2f[:], rr2[:])
                        ir2 = rrp.tile([33, QB], F16, tag="ir2")
                        nc.vector.tensor_copy(ir2[:], ir2f[:])
                        return ir2

                    def normalize(p, ous_l, j_l, ctxl_l, pbpool, ir2=None):
                        # PE tile-broadcast of the reciprocals per head half
                        if ir2 is None:
                            ir2 = make_ir2(ous_l[p])
                        pb = pbpool.tile(
                            [128, QB],
                            F32,
                            tag="po" if pbpool is pop else "proj",
                            name="pb",
                        )
                        nc.tensor.matmul(
                            pb[0:64, :],
                            onehot2[0:1, 0:64],
                            ir2[0:1, :],
                            start=True,
                            stop=True,
                            tile_position=(0, 0),
                        )
                        nc.tensor.matmul(
                            pb[64:128, :],
                            onehot2[32:33, 64:128],
                            ir2[32:33, :],
                            start=True,
                            stop=True,
                            tile_position=(32, 64),
                        )
                        for hh in range(2):
                            nc.vector.tensor_mul(
                                ctxl_l[
                                    64 * hh : 64 * hh + 64,
                                    p,
                                    j_l * QB : (j_l + 1) * QB,
                                ],
                                ous_l[p][0:64, hh * QB : (hh + 1) * QB],
                                pb[64 * hh : 64 * hh + 64, :],
                            )

                    pre = None
                    if b == 0 and j == 0:
                        # first block: wq lands before wkv on the scalar queue,
                        # so keep the Q-first order for the DMA cascade
                        q_proj(0)
                        q_proj(1)
                        vt_sb = kv_proj()
                        v_transp(vt_sb)
                    else:
                        # steady state: K first so the kt->score->exp chain of
                        # pair 0 fills while Q pair 1 still streams on the PE
                        vt_sb = kv_proj()
                        q_proj(0)
                        pre = [emit_scores(0, 0), emit_scores(1, 0)]
                        q_proj(1)
                        v_transp(vt_sb)
                    # the previous block's deferred pair-1 normalize lands
                    # here, AFTER this block's critical kt/qt copies and
                    # pre-seeded scores, so it no longer blocks them on DVE
                    for fn in defer_norm:
                        fn()
                    defer_norm.clear()

                    for _ in range(4):
                        emit_fp_unit()

                    # attention for this query block
                    ous = []
                    for p in range(PAIRS):
                        po = pop.tile([65, 2 * QB], F32, tag="po")
                        if p == 0 and pre is not None:
                            pipe = pre
                        else:
                            pipe = [emit_scores(0, p)]
                            if nkk > 1:
                                pipe.append(emit_scores(1, p))
                        for kk in range(nkk):
                            es_cur, q0 = pipe.pop(0)
                            if kk + 2 < nkk:
                                pipe.append(emit_scores(kk + 2, p))
                            for hh in range(2):
                                nc.tensor.matmul(
                                    po[0:65, hh * QB + q0 : (hh + 1) * QB],
                                    vv[:, kk * VW : (kk + 1) * VW],
                                    es_cur[:, hh * QB + q0 : (hh + 1) * QB],
                                    start=(kk == 0),
                                    stop=(kk == nkk - 1),
                                )
                        # single-copy PSUM drain: rows 0:64 ctx, row 64 denom
                        ou = oup.tile([65, 2 * QB], F32, tag="ou")
                        nc.vector.tensor_copy(ou[:], po[:])
                        ous.append(ou)
                        if last_j:
                            # pb from the projection pool: no aliasing wait on
                            # po, so the tail output projection starts early
                            normalize(p, ous, j, ctxl, prjp)

                    if not last_j:
                        normalize(0, ous, j, ctxl, pop)
                        defer_norm.append(
                            lambda o=ous, jj=j, cl=ctxl: normalize(1, o, jj, cl, pop)
                        )

                    fp_units.extend((b, j, och) for och in range(4))
            while fp_units:
                emit_fp_unit()
    nc.finalize()
    return nc


def _prep_in_maps(x, wq, wk, wv, wo):
    # strip-major x: [B, NJ, 128, ND, QB], 16KB contiguous per partition/strip
    xT = (
        x.transpose(0, 2, 1)
        .reshape(B, ND, 128, NJ, QB)
        .transpose(0, 3, 2, 1, 4)
        .astype(np.float16)
    )
    xT = np.ascontiguousarray(xT)
    k_idx = np.arange(128)[:, None]
    q_idx = np.arange(QB)[None, :]
    masks = np.stack(
        [(128 * m + k_idx <= q_idx).astype(np.float16) for m in range(4)]
    )
    oh2 = np.zeros((33, 128), np.float16)
    oh2[0, 0:64] = 1.0
    oh2[32, 64:128] = 1.0
    # single 128x128 diagonal-square mask (same for every diagonal tile)
    masks_h = np.ascontiguousarray(masks[0, :, 0:128].reshape(128, 1, 128))

    def pack_pnm(w):  # [D_in, M] -> [128 p, ND n, M]
        return np.ascontiguousarray(
            w.reshape(ND, 128, w.shape[1]).transpose(1, 0, 2)
        )

    in_maps = []
    for c in range(NC):
        wq_c = (wq[c * 256 : (c + 1) * 256] * np.float32(DH ** -0.5)).astype(np.float16)
        wkv_c = np.concatenate(
            [wk[c * DH : (c + 1) * DH], wv[c * DH : (c + 1) * DH]], axis=0
        ).astype(np.float16)
        wo_c = wo[:, c * 256 : (c + 1) * 256].astype(np.float16)  # [2048, 256]
        woT_c = wo_c.T  # [256, 2048]
        wo_h = np.ascontiguousarray(
            woT_c.reshape(2, 128, D).transpose(1, 0, 2)
        )  # [128 p, 2 g, D]
        in_maps.append(
            {
                "xT": xT,
                "wqT": pack_pnm(np.ascontiguousarray(wq_c.T)),
                "wkvT": pack_pnm(np.ascontiguousarray(wkv_c.T)),
                "woT": wo_h,
                "masks": masks_h,
                "oh2": oh2,
            }
        )
    return in_maps


def run(inputs, trace=False, trace_kwargs=None):
    if "nc" not in _CACHE:
        _CACHE["nc"] = _build()
    nc = _CACHE["nc"]
    in_maps = _prep_in_maps(
        np.asarray(inputs["x"], np.float32),
        np.asarray(inputs["wq"], np.float32),
        np.asarray(inputs["wk"], np.float32),
        np.asarray(inputs["wv"], np.float32),
        np.asarray(inputs["wo"], np.float32),
    )
    res = run_bass_kernel_spmd(
        nc,
        in_maps,
        core_ids=list(range(NC)),
        trace=trace,
        **(trace_kwargs or {}),
    )
    acc = np.zeros((B, NJ, 4, 128, 4, QB), np.float32)
    for r in res.results:
        acc += r["outT"]
    # [B, tt, och, p, n, q] -> [B, d=(och,n,p), t=(tt,q)] -> [B, T, D]
    full = acc.transpose(0, 2, 4, 3, 1, 5).reshape(B, D, T).transpose(0, 2, 1)
    return np.ascontiguousarray(full), res


def kernel(**inputs) -> np.ndarray:
    out, _ = run(inputs, trace=False)
    return out
